# revision 11
# baseline (speedup 1.0000x reference)
"""Nose-Hoover checkpointed integrator on 8 Trainium2 cores.

Data-parallel: 4096 systems sharded as 512 systems/core. All state lives in
SBUF; each core integrates its shard for n_steps and DMAs (x, v) snapshots to
DRAM every store_every steps.

Layout per core: [128 partitions = systems (s mod 128), free = G groups of 64
dof] with group g = s // 128 (G = 4). Per-system scalars (alpha/exp factors,
v^2 sums) live as [128, G] tiles.

Math (force = -x, harmonic):
  beta := -(DT/2)*alpha, so thermostat factor f = exp(beta).
  beta update: beta += s_c*(v2 - e), s_c = -DT^2/(8Q), e = ndof*kT.
  v2 is carried: after a uniform scale v *= f, v2 scales by f^2 (no re-reduce).
  The end-of-step scale f' and next step's start scale f are fused into one
  multiply gf = f'*f; on snapshot steps v*f' is materialized separately.
"""

import numpy as np

DT = 0.001
N_CORES = 8
P = 128

_BUILD_CACHE = {}


def _split_multi_waits(nc, mybir):
    """This container's walrus encodes at most one sem-wait per instruction;
    hoist extra waits onto single-wait NoOps on the same engine."""
    for f in nc.m.functions:
        for bb in f.blocks:
            out = []
            for inst in bb.instructions:
                si = inst.sync_info
                if si is not None and len(si.on_wait) > 1:
                    waits = list(si.on_wait)
                    for w in waits[:-1]:
                        out.append(
                            mybir.InstNoOp(
                                name=nc.get_next_instruction_name(),
                                sync_info=mybir.SyncInfo(on_wait=[w], on_update=[]),
                                bass_nofuse=True,
                                engine=inst.engine,
                            )
                        )
                    inst.sync_info = mybir.SyncInfo(
                        on_wait=[waits[-1]], on_update=list(si.on_update)
                    )
                out.append(inst)
            bb.instructions = out


def _build(B_core, D, n_steps, store_every, kT, mass, Q, skip_dma=False, bench_iters=None):
    import concourse.bass as bass
    import concourse.mybir as mybir
    from concourse.tile import TileContext

    G = B_core // P
    FD = G * D
    n_chunks = n_steps // store_every
    rem_steps = n_steps - n_chunks * store_every
    if bench_iters is not None:
        n_chunks, rem_steps = 1, 0

    k = DT / (2.0 * mass)
    e = float(D) * kT
    s_c = -(DT * DT) / (8.0 * Q)
    mdt2 = -DT / 2.0

    AF = mybir.ActivationFunctionType
    OP = mybir.AluOpType
    f32 = mybir.dt.float32

    nc = bass.Bass()

    # Register const-AP tiles for the per-position Exp biases (the activation
    # bias operand must be a [128,1] SBUF constant).
    q_bias = -s_c * e

    def reg_const(val):
        key = (f32, float(val))
        if key not in nc.const_aps.aps:
            t = nc.alloc_sbuf_tensor(
                f"constb-{len(nc.const_aps.aps)}", [128, 1], f32
            )
            nc.gpsimd.memset(t.ap(), float(val))
            nc.const_aps.aps[key] = t.ap()

    for p_pos in range(max(store_every, n_steps - (n_steps // store_every) * store_every)):
        for u in (4 * p_pos + 1, 4 * p_pos + 3):
            reg_const(u * q_bias)
            reg_const(2 * u * q_bias)
    nc.all_engine_barrier()

    x0 = nc.dram_tensor("x0", [B_core, D], f32, kind="ExternalInput")
    v0 = nc.dram_tensor("v0", [B_core, D], f32, kind="ExternalInput")
    a0 = nc.dram_tensor("alpha0", [B_core], f32, kind="ExternalInput")
    out_x = nc.dram_tensor("out_x", [n_chunks, G, P, D], f32, kind="ExternalOutput")
    out_v = nc.dram_tensor("out_v", [n_chunks, G, P, D], f32, kind="ExternalOutput")

    def gs(g):
        return slice(g * D, (g + 1) * D)

    with TileContext(nc) as tc:
        with (
            tc.tile_pool(name="state", bufs=1) as state,
            tc.tile_pool(name="stage", bufs=3) as stage,
        ):
            X = state.tile([P, FD], f32, tag="X")
            V = state.tile([P, FD], f32, tag="V")
            SQ = state.tile([P, FD], f32, tag="SQ")
            R = state.tile([P, G], f32, tag="R")
            BETA = state.tile([P, G], f32, tag="BETA")
            F = state.tile([P, G], f32, tag="F")
            FP = state.tile([P, G], f32, tag="FP")
            GF = state.tile([P, G], f32, tag="GF")
            T1 = state.tile([P, G], f32, tag="T1")
            F2 = state.tile([P, G], f32, tag="F2")

            nc.sync.dma_start(
                out=X[:].rearrange("p (g d) -> p g d", g=G),
                in_=x0[:].rearrange("(g p) d -> p g d", p=P),
            )
            nc.sync.dma_start(
                out=V[:].rearrange("p (g d) -> p g d", g=G),
                in_=v0[:].rearrange("(g p) d -> p g d", p=P),
            )
            nc.sync.dma_start(out=T1[:], in_=a0[:].rearrange("(g p) -> p g", p=P))
            nc.vector.tensor_scalar(BETA[:], T1[:], mdt2, None, OP.mult)
            nc.vector.memset(FP[:], 1.0)
            for g in range(G):
                nc.scalar.activation(
                    out=SQ[:, gs(g)],
                    in_=V[:, gs(g)],
                    func=AF.Square,
                    accum_out=R[:, g : g + 1],
                )

            # BETA carries beta-tilde: only the s_c*v2 parts of each alpha
            # update. The -s_c*e offsets are compile-time per-position and are
            # folded into the Exp instruction biases (true beta after u
            # updates = beta_tilde + u*q). Each chunk ends with a renorm
            # (BETA += 4*store_every*q) so the loop body is iteration-
            # invariant.
            q = -s_c * e

            def beta_update():
                # beta_tilde += s_c * R
                nc.vector.scalar_tensor_tensor(
                    BETA[:], R[:], s_c, BETA[:], OP.mult, OP.add
                )

            def step(p, snap_ci):
                # Incoming: R = sum(v_c^2) of the previous step scaled by its
                # trailing thermostat factor^2 (i.e. r4 = f'^2 * r3); FP = f'
                # of the previous step (not yet applied to V).
                u1 = 4 * p + 1
                u3 = 4 * p + 3
                beta_update()  # alpha update 1
                nc.scalar.activation(out=F[:], in_=BETA[:], func=AF.Exp, bias=u1 * q)
                nc.scalar.activation(
                    out=F2[:], in_=BETA[:], func=AF.Exp, scale=2.0, bias=2 * u1 * q
                )
                nc.vector.tensor_tensor(R[:], R[:], F2[:], OP.mult)
                beta_update()  # alpha update 2
                # fused scale: previous step's f' and this step's f
                nc.vector.tensor_tensor(GF[:], FP[:], F[:], OP.mult)
                for g in range(G):
                    nc.vector.tensor_scalar(
                        V[:, gs(g)], V[:, gs(g)], GF[:, g : g + 1], None, OP.mult
                    )
                # kick-drift-kick (force = -x)
                nc.vector.scalar_tensor_tensor(V[:], X[:], -k, V[:], OP.mult, OP.add)
                nc.vector.scalar_tensor_tensor(X[:], V[:], DT, X[:], OP.mult, OP.add)
                nc.vector.scalar_tensor_tensor(V[:], X[:], -k, V[:], OP.mult, OP.add)
                for g in range(G):
                    nc.scalar.activation(
                        out=SQ[:, gs(g)],
                        in_=V[:, gs(g)],
                        func=AF.Square,
                        accum_out=R[:, g : g + 1],
                    )
                beta_update()  # alpha update 3
                nc.scalar.activation(out=FP[:], in_=BETA[:], func=AF.Exp, bias=u3 * q)
                nc.scalar.activation(
                    out=F2[:], in_=BETA[:], func=AF.Exp, scale=2.0, bias=2 * u3 * q
                )
                if snap_ci is not None and not skip_dma:
                    XS = stage.tile([P, FD], f32, tag="XS")
                    VS = stage.tile([P, FD], f32, tag="VS")
                    nc.gpsimd.tensor_copy(XS[:], X[:])
                    for g in range(G):
                        nc.gpsimd.tensor_scalar(
                            VS[:, gs(g)], V[:, gs(g)], FP[:, g : g + 1], None, OP.mult
                        )
                    nc.sync.dma_start(
                        out=out_x[snap_ci, :, :, :].rearrange("o g p d -> (o p) g d"),
                        in_=XS[:].rearrange("p (g d) -> p g d", g=G),
                    )
                    nc.sync.dma_start(
                        out=out_v[snap_ci, :, :, :].rearrange("o g p d -> (o p) g d"),
                        in_=VS[:].rearrange("p (g d) -> p g d", g=G),
                    )
                nc.vector.tensor_tensor(R[:], R[:], F2[:], OP.mult)
                beta_update()  # alpha update 4

            def chunk_renorm(n_in_chunk):
                nc.vector.tensor_scalar(
                    BETA[:], BETA[:], 4.0 * n_in_chunk * q, None, OP.add
                )

            n_loop = n_chunks if bench_iters is None else bench_iters
            if n_loop > 0:
                with tc.For_i(
                    0, n_loop, hint_engines=(mybir.EngineType.DVE,)
                ) as ci:
                    ci_slot = bass.ds(ci, 1) if bench_iters is None else bass.ds(ci * 0, 1)
                    for p in range(store_every - 1):
                        step(p, None)
                    step(store_every - 1, ci_slot)
                    chunk_renorm(store_every)
            for p in range(rem_steps):
                step(p, None)

    _split_multi_waits(nc, mybir)
    return nc


def kernel(x0, v0, alpha0, kT, mass, Q, n_steps, store_every):
    from concourse.bass_utils import run_bass_kernel_spmd

    x0 = np.asarray(x0, dtype=np.float32)
    v0 = np.asarray(v0, dtype=np.float32)
    alpha0 = np.asarray(alpha0, dtype=np.float32)
    kT_f, mass_f, Q_f = float(np.asarray(kT)), float(np.asarray(mass)), float(np.asarray(Q))
    n_steps = int(np.asarray(n_steps))
    store_every = int(np.asarray(store_every))

    B, D = x0.shape
    B_core = B // N_CORES
    n_chunks = n_steps // store_every

    key = (B_core, D, n_steps, store_every, kT_f, mass_f, Q_f)
    if key not in _BUILD_CACHE:
        _BUILD_CACHE[key] = _build(B_core, D, n_steps, store_every, kT_f, mass_f, Q_f)
    nc = _BUILD_CACHE[key]

    in_maps = []
    for c in range(N_CORES):
        sl = slice(c * B_core, (c + 1) * B_core)
        in_maps.append(
            {
                "x0": np.ascontiguousarray(x0[sl]),
                "v0": np.ascontiguousarray(v0[sl]),
                "alpha0": np.ascontiguousarray(alpha0[sl]),
            }
        )

    res = run_bass_kernel_spmd(nc, in_maps, core_ids=list(range(N_CORES)))
    results = res.results

    traj_x = np.empty((n_chunks + 1, B, D), np.float32)
    traj_v = np.empty((n_chunks + 1, B, D), np.float32)
    traj_x[0] = x0
    traj_v[0] = v0
    for c in range(N_CORES):
        sl = slice(c * B_core, (c + 1) * B_core)
        traj_x[1:, sl] = results[c]["out_x"].reshape(n_chunks, B_core, D)
        traj_v[1:, sl] = results[c]["out_v"].reshape(n_chunks, B_core, D)
    return traj_x, traj_v
